# revision 20
# baseline (speedup 1.0000x reference)
"""Chamfer distance kernel v3 for Trainium2 (Bass/Tile), SPMD over 8 NeuronCores.

Structure vs v2 (171us):
  - Host pre-prep: x/y are transposed, scaled (-2x), cast to f16 and the
    norm rows (x2/y2) computed in numpy; uploaded directly in the K-major
    layout the PE wants. Kills all phase-0 PE transposes/converts/squares.
  - Norms baked into the matmul: lhsT has K=66 rows (-2x^T | ones | x2),
    rhs has (y^T | y2 | ones), so psum holds the full squared distance and
    the ACT drain is a pure f32->f16 copy (no bias adds).
  - Per tile: ACT drains 2x[128,2048] psum halves, DVE does one fold-min
    (row path, shipped 2048-wide to host) and one colacc min.
"""

import sys

if "/opt/trn_rl_repo" not in sys.path:
    sys.path.insert(0, "/opt/trn_rl_repo")

import numpy as np

B = 8
N = 4096
M = 4096
K = 64
KA = K + 2  # -2x rows | ones | x2   (lhsT);  y rows | y2 | ones  (rhs)
NT = 128    # x rows per tile
MT = 512    # rhs cols per matmul
WS = 128    # drain columns offloaded from ACT to DVE per tile

_COMPILED = {}
LAST_RESULTS = None


def _build(n_rows, m_cols, num_cores):
    import concourse.bacc as bacc
    import concourse.mybir as mybir
    import concourse.tile as tile

    f32 = mybir.dt.float32
    f16 = mybir.dt.float16
    AX = mybir.AxisListType
    OP = mybir.AluOpType

    n_nt = n_rows // NT  # 32 tiles

    nc = bacc.Bacc(
        "TRN2", target_bir_lowering=False, debug=False, num_devices=num_cores
    )
    xtd = nc.dram_tensor("xt", [KA, n_rows], f16, kind="ExternalInput")
    ytd = nc.dram_tensor("yt", [KA, m_cols], f16, kind="ExternalInput")
    rowp = nc.dram_tensor("rowp", [n_rows, 2048], f16, kind="ExternalOutput")
    outc = nc.dram_tensor("outc", [128, m_cols], f16, kind="ExternalOutput")

    with tile.TileContext(nc) as tc:
        with (
            tc.tile_pool(name="const", bufs=1) as cpool,
            tc.tile_pool(name="mpsum", bufs=2, space="PSUM") as ps_pool,
            tc.tile_pool(name="tsbp", bufs=3) as tsb_pool,
            tc.tile_pool(name="scrp", bufs=4) as scr_pool,
        ):
            XT = cpool.tile([KA, n_rows], f16, name="XT")
            YT = cpool.tile([KA, m_cols], f16, name="YT")
            colacc = cpool.tile([128, m_cols], f16, name="colacc")

            # load K-major operands (norm/ones rows prebuilt on host).
            # Fine-grained chunks alternating across two queues so the first
            # matmuls start early and never starve.
            nc.sync.dma_start(YT[:, 0:512], ytd[:, 0:512])
            nc.gpsimd.dma_start(XT[:, 0:1024], xtd[:, 0:1024])
            nc.sync.dma_start(YT[:, 512:1024], ytd[:, 512:1024])
            nc.gpsimd.dma_start(YT[:, 1024:2048], ytd[:, 1024:2048])
            nc.sync.dma_start(YT[:, 2048:3072], ytd[:, 2048:3072])
            nc.gpsimd.dma_start(YT[:, 3072:4096], ytd[:, 3072:4096])
            nc.sync.dma_start(XT[:, 1024:4096], xtd[:, 1024:4096])

            for t in range(n_nt):
                lhsT = XT[:, t * NT : (t + 1) * NT]
                last = t == n_nt - 1
                tsb = tsb_pool.tile([128, m_cols], f16, tag="tsb", name="tsb")
                for half in range(2):
                    ps = ps_pool.tile([128, 2048], f32, tag="ps", name="ps")
                    for h in range(2048 // MT):
                        c0 = half * 2048 + h * MT
                        nc.tensor.matmul(
                            ps[:, h * MT : (h + 1) * MT],
                            lhsT=lhsT,
                            rhs=YT[:, c0 : c0 + MT],
                            start=True,
                            stop=True,
                        )
                    # drain: psum f32 -> sbuf f16 (full squared distance).
                    # DVE takes the first W cols of half 1 (it only waits on
                    # the first matmul of the half), ACT drains the rest.
                    hs = slice(half * 2048, (half + 1) * 2048)
                    if half == 1:
                        nc.vector.tensor_scalar_mul(
                            tsb[:, 2048 : 2048 + WS], ps[:, 0:WS], 1.0
                        )
                        nc.scalar.copy(tsb[:, 2048 + WS : 4096], ps[:, WS:2048])
                    else:
                        nc.scalar.copy(tsb[:, hs], ps)

                    # col path per half: starts right after this drain
                    if t == 0:
                        nc.vector.tensor_copy(colacc[:, hs], tsb[:, hs])
                    elif last:
                        q = 1024
                        for jj in range(2):
                            j = half * 2 + jj
                            nc.vector.tensor_tensor(
                                colacc[:, j * q : (j + 1) * q],
                                tsb[:, j * q : (j + 1) * q],
                                colacc[:, j * q : (j + 1) * q],
                                OP.min,
                            )
                            qeng = nc.sync if j % 2 == 0 else nc.gpsimd
                            qeng.dma_start(
                                outc[:, j * q : (j + 1) * q],
                                colacc[:, j * q : (j + 1) * q],
                            )
                    else:
                        nc.vector.tensor_tensor(
                            colacc[:, hs], tsb[:, hs], colacc[:, hs], OP.min
                        )

                # row path: fold 4096 -> 2048, ship to host
                scr = scr_pool.tile([128, 2048], f16, tag="scr", name="scr")
                nc.vector.tensor_tensor(
                    scr, tsb[:, 0:2048], tsb[:, 2048:4096], OP.min
                )
                eng = nc.gpsimd if t >= n_nt - 3 else nc.sync
                eng.dma_start(rowp[t * 128 : (t + 1) * 128, :], scr)

    nc.compile()
    return nc


def _get(n_rows, m_cols, num_cores):
    key = (n_rows, m_cols, num_cores)
    if key not in _COMPILED:
        _COMPILED[key] = _build(n_rows, m_cols, num_cores)
    return _COMPILED[key]


def _prep(x, y):
    # host-side layout prep: K-major f16 operands with ones/norm rows baked
    n = x.shape[0]
    m = y.shape[0]
    xt = np.empty((KA, n), dtype=np.float16)
    xt[0:K] = (-2.0 * x.T).astype(np.float16)
    xt[K] = 1.0
    xt[K + 1] = (x.astype(np.float64) ** 2).sum(axis=1).astype(np.float16)
    yt = np.empty((KA, m), dtype=np.float16)
    yt[0:K] = y.T.astype(np.float16)
    yt[K] = (y.astype(np.float64) ** 2).sum(axis=1).astype(np.float16)
    yt[K + 1] = 1.0
    return {"xt": np.ascontiguousarray(xt), "yt": np.ascontiguousarray(yt)}


def _run(x, y, n_rows, m_cols, num_cores, trace=False):
    global LAST_RESULTS
    from concourse import bass_utils

    nc = _get(n_rows, m_cols, num_cores)
    in_maps = [_prep(x[b], y[b]) for b in range(num_cores)]
    res = bass_utils.run_bass_kernel_spmd(
        nc, in_maps, core_ids=list(range(num_cores)), trace=trace
    )
    LAST_RESULTS = res
    return [(r["rowp"], r["outc"]) for r in res.results]


def _postprocess(outs):
    total = 0.0
    for rowpart, colacc in outs:
        rmin = rowpart.astype(np.float32).min(axis=1)
        colmin = colacc.astype(np.float32).min(axis=0)
        d1 = np.sqrt(np.maximum(rmin.astype(np.float64), 0.0)).mean()
        d0 = np.sqrt(np.maximum(colmin.astype(np.float64), 0.0)).mean()
        total += d0 + d1
    return np.float32(total / len(outs))


def kernel(input1, input2):
    x = np.asarray(input1, dtype=np.float32)
    y = np.asarray(input2, dtype=np.float32)
    assert x.shape == (B, N, K) and y.shape == (B, M, K), (x.shape, y.shape)
    outs = _run(x, y, N, M, B)
    return _postprocess(outs)


# revision 22
# speedup vs baseline: 1.2151x; 1.2151x over previous
"""Chamfer distance kernel v3 for Trainium2 (Bass/Tile), SPMD over 8 NeuronCores.

Structure vs v2 (171us):
  - Host pre-prep: x/y are transposed, scaled (-2x), cast to f16 and the
    norm rows (x2/y2) computed in numpy; uploaded directly in the K-major
    layout the PE wants. Kills all phase-0 PE transposes/converts/squares.
  - Norms baked into the matmul: lhsT has K=66 rows (-2x^T | ones | x2),
    rhs has (y^T | y2 | ones), so psum holds the full squared distance and
    the ACT drain is a pure f32->f16 copy (no bias adds).
  - Per tile: ACT drains 2x[128,2048] psum halves, DVE does one fold-min
    (row path, shipped 2048-wide to host) and one colacc min.
"""

import sys

if "/opt/trn_rl_repo" not in sys.path:
    sys.path.insert(0, "/opt/trn_rl_repo")

import numpy as np

B = 8
N = 4096
M = 4096
K = 64
KA = K + 2  # -2x rows | ones | x2   (lhsT);  y rows | y2 | ones  (rhs)
NT = 128    # x rows per tile
MT = 512    # rhs cols per matmul

_COMPILED = {}
LAST_RESULTS = None


def _build(n_rows, m_cols, num_cores):
    import concourse.bacc as bacc
    import concourse.mybir as mybir
    import concourse.tile as tile

    f32 = mybir.dt.float32
    f16 = mybir.dt.float16
    AX = mybir.AxisListType
    OP = mybir.AluOpType

    n_nt = n_rows // NT  # 32 tiles

    nc = bacc.Bacc(
        "TRN2", target_bir_lowering=False, debug=False, num_devices=num_cores
    )
    xtd = nc.dram_tensor("xt", [KA, n_rows], f16, kind="ExternalInput")
    ytd = nc.dram_tensor("yt", [KA, m_cols], f16, kind="ExternalInput")
    rowp = nc.dram_tensor("rowp", [n_rows, 2048], f16, kind="ExternalOutput")
    outc = nc.dram_tensor("outc", [128, m_cols], f16, kind="ExternalOutput")

    with tile.TileContext(nc) as tc:
        with (
            tc.tile_pool(name="const", bufs=1) as cpool,
            tc.tile_pool(name="mpsum", bufs=2, space="PSUM") as ps_pool,
            tc.tile_pool(name="tsbp", bufs=3) as tsb_pool,
            tc.tile_pool(name="scrp", bufs=4) as scr_pool,
        ):
            XT = cpool.tile([KA, n_rows], f16, name="XT")
            YT = cpool.tile([KA, m_cols], f16, name="YT")
            colacc = cpool.tile([128, m_cols], f16, name="colacc")

            # load K-major operands (norm/ones rows prebuilt on host).
            # Fine-grained chunks alternating across two queues so the first
            # matmuls start early and never starve.
            nc.sync.dma_start(YT[:, 0:512], ytd[:, 0:512])
            nc.gpsimd.dma_start(XT[:, 0:1024], xtd[:, 0:1024])
            nc.sync.dma_start(YT[:, 512:1024], ytd[:, 512:1024])
            nc.gpsimd.dma_start(YT[:, 1024:2048], ytd[:, 1024:2048])
            nc.sync.dma_start(YT[:, 2048:3072], ytd[:, 2048:3072])
            nc.gpsimd.dma_start(YT[:, 3072:4096], ytd[:, 3072:4096])
            nc.sync.dma_start(XT[:, 1024:4096], xtd[:, 1024:4096])

            for t in range(n_nt):
                lhsT = XT[:, t * NT : (t + 1) * NT]
                last = t == n_nt - 1
                tsb = tsb_pool.tile([128, m_cols], f16, tag="tsb", name="tsb")
                for half in range(2):
                    ps = ps_pool.tile([128, 2048], f32, tag="ps", name="ps")
                    for h in range(2048 // MT):
                        c0 = half * 2048 + h * MT
                        nc.tensor.matmul(
                            ps[:, h * MT : (h + 1) * MT],
                            lhsT=lhsT,
                            rhs=YT[:, c0 : c0 + MT],
                            start=True,
                            stop=True,
                        )
                    # drain: psum f32 -> sbuf f16 (full squared distance)
                    hs = slice(half * 2048, (half + 1) * 2048)
                    nc.scalar.copy(tsb[:, hs], ps)

                    # col path per half: starts right after this drain
                    if t == 0:
                        nc.vector.tensor_copy(colacc[:, hs], tsb[:, hs])
                    elif last:
                        q = 1024
                        for jj in range(2):
                            j = half * 2 + jj
                            nc.vector.tensor_tensor(
                                colacc[:, j * q : (j + 1) * q],
                                tsb[:, j * q : (j + 1) * q],
                                colacc[:, j * q : (j + 1) * q],
                                OP.min,
                            )
                            qeng = nc.sync if j % 2 == 0 else nc.gpsimd
                            qeng.dma_start(
                                outc[:, j * q : (j + 1) * q],
                                colacc[:, j * q : (j + 1) * q],
                            )
                    else:
                        nc.vector.tensor_tensor(
                            colacc[:, hs], tsb[:, hs], colacc[:, hs], OP.min
                        )

                # row path: fold 4096 -> 2048, ship to host
                scr = scr_pool.tile([128, 2048], f16, tag="scr", name="scr")
                nc.vector.tensor_tensor(
                    scr, tsb[:, 0:2048], tsb[:, 2048:4096], OP.min
                )
                eng = nc.gpsimd if t >= n_nt - 3 else nc.sync
                eng.dma_start(rowp[t * 128 : (t + 1) * 128, :], scr)

    nc.compile()
    return nc


def _get(n_rows, m_cols, num_cores):
    key = (n_rows, m_cols, num_cores)
    if key not in _COMPILED:
        _COMPILED[key] = _build(n_rows, m_cols, num_cores)
    return _COMPILED[key]


def _prep(x, y):
    # host-side layout prep: K-major f16 operands with ones/norm rows baked
    n = x.shape[0]
    m = y.shape[0]
    xt = np.empty((KA, n), dtype=np.float16)
    xt[0:K] = (-2.0 * x.T).astype(np.float16)
    xt[K] = 1.0
    xt[K + 1] = (x.astype(np.float64) ** 2).sum(axis=1).astype(np.float16)
    yt = np.empty((KA, m), dtype=np.float16)
    yt[0:K] = y.T.astype(np.float16)
    yt[K] = (y.astype(np.float64) ** 2).sum(axis=1).astype(np.float16)
    yt[K + 1] = 1.0
    return {"xt": np.ascontiguousarray(xt), "yt": np.ascontiguousarray(yt)}


def _run(x, y, n_rows, m_cols, num_cores, trace=False):
    global LAST_RESULTS
    from concourse import bass_utils

    nc = _get(n_rows, m_cols, num_cores)
    in_maps = [_prep(x[b], y[b]) for b in range(num_cores)]
    res = bass_utils.run_bass_kernel_spmd(
        nc, in_maps, core_ids=list(range(num_cores)), trace=trace
    )
    LAST_RESULTS = res
    return [(r["rowp"], r["outc"]) for r in res.results]


def _postprocess(outs):
    total = 0.0
    for rowpart, colacc in outs:
        rmin = rowpart.astype(np.float32).min(axis=1)
        colmin = colacc.astype(np.float32).min(axis=0)
        d1 = np.sqrt(np.maximum(rmin.astype(np.float64), 0.0)).mean()
        d0 = np.sqrt(np.maximum(colmin.astype(np.float64), 0.0)).mean()
        total += d0 + d1
    return np.float32(total / len(outs))


def kernel(input1, input2):
    x = np.asarray(input1, dtype=np.float32)
    y = np.asarray(input2, dtype=np.float32)
    assert x.shape == (B, N, K) and y.shape == (B, M, K), (x.shape, y.shape)
    outs = _run(x, y, N, M, B)
    return _postprocess(outs)


# revision 25
# speedup vs baseline: 1.2152x; 1.0000x over previous
"""Chamfer distance kernel v3 for Trainium2 (Bass/Tile), SPMD over 8 NeuronCores.

Structure vs v2 (171us):
  - Host pre-prep: x/y are transposed, scaled (-2x), cast to f16 and the
    norm rows (x2/y2) computed in numpy; uploaded directly in the K-major
    layout the PE wants. Kills all phase-0 PE transposes/converts/squares.
  - Norms baked into the matmul: lhsT has K=66 rows (-2x^T | ones | x2),
    rhs has (y^T | y2 | ones), so psum holds the full squared distance and
    the ACT drain is a pure f32->f16 copy (no bias adds).
  - Per tile: ACT drains 2x[128,2048] psum halves, DVE does one fold-min
    (row path, shipped 2048-wide to host) and one colacc min.
"""

import sys

if "/opt/trn_rl_repo" not in sys.path:
    sys.path.insert(0, "/opt/trn_rl_repo")

import numpy as np

B = 8
N = 4096
M = 4096
K = 64
KA = K + 2  # -2x rows | ones | x2   (lhsT);  y rows | y2 | ones  (rhs)
NT = 128    # x rows per tile
MT = 512    # rhs cols per matmul

_COMPILED = {}
LAST_RESULTS = None


def _build(n_rows, m_cols, num_cores):
    import concourse.bacc as bacc
    import concourse.mybir as mybir
    import concourse.tile as tile

    f32 = mybir.dt.float32
    f16 = mybir.dt.float16
    AX = mybir.AxisListType
    OP = mybir.AluOpType

    n_nt = n_rows // NT  # 32 tiles

    nc = bacc.Bacc(
        "TRN2", target_bir_lowering=False, debug=False, num_devices=num_cores
    )
    xtd = nc.dram_tensor("xt", [KA, n_rows], f16, kind="ExternalInput")
    ytd = nc.dram_tensor("yt", [KA, m_cols], f16, kind="ExternalInput")
    rowp = nc.dram_tensor("rowp", [n_rows, 2048], f16, kind="ExternalOutput")
    outc = nc.dram_tensor("outc", [128, m_cols], f16, kind="ExternalOutput")

    with tile.TileContext(nc) as tc:
        with (
            tc.tile_pool(name="const", bufs=1) as cpool,
            tc.tile_pool(name="mpsum", bufs=2, space="PSUM") as ps_pool,
            tc.tile_pool(name="tsbp", bufs=3) as tsb_pool,
            tc.tile_pool(name="scrp", bufs=4) as scr_pool,
        ):
            XT = cpool.tile([KA, n_rows], f16, name="XT")
            YT = cpool.tile([KA, m_cols], f16, name="YT")
            colacc = cpool.tile([128, m_cols], f16, name="colacc")

            # load K-major operands (norm/ones rows prebuilt on host).
            # Fine-grained chunks alternating across two queues so the first
            # matmuls start early and never starve.
            nc.sync.dma_start(YT[:, 0:256], ytd[:, 0:256])
            nc.gpsimd.dma_start(XT[:, 0:1024], xtd[:, 0:1024])
            nc.sync.dma_start(YT[:, 256:1024], ytd[:, 256:1024])
            nc.gpsimd.dma_start(YT[:, 1024:2048], ytd[:, 1024:2048])
            nc.sync.dma_start(YT[:, 2048:3072], ytd[:, 2048:3072])
            nc.gpsimd.dma_start(YT[:, 3072:4096], ytd[:, 3072:4096])
            nc.sync.dma_start(XT[:, 1024:4096], xtd[:, 1024:4096])

            for t in range(n_nt):
                lhsT = XT[:, t * NT : (t + 1) * NT]
                last = t == n_nt - 1
                tsb = tsb_pool.tile([128, m_cols], f16, tag="tsb", name="tsb")
                for half in range(2):
                    ps = ps_pool.tile([128, 2048], f32, tag="ps", name="ps")
                    for h in range(2048 // MT):
                        c0 = half * 2048 + h * MT
                        nc.tensor.matmul(
                            ps[:, h * MT : (h + 1) * MT],
                            lhsT=lhsT,
                            rhs=YT[:, c0 : c0 + MT],
                            start=True,
                            stop=True,
                        )
                    # drain: psum f32 -> sbuf f16 (full squared distance)
                    hs = slice(half * 2048, (half + 1) * 2048)
                    nc.scalar.copy(tsb[:, hs], ps)

                    # after the half-1 drain, fold first so the rowp DMA
                    # starts early; the col h1 update has a full tile of slack
                    if half == 1 and 0 < t < n_nt - 1:
                        scr = scr_pool.tile([128, 2048], f16, tag="scr", name="scr")
                        nc.vector.tensor_tensor(
                            scr, tsb[:, 0:2048], tsb[:, 2048:4096], OP.min
                        )
                        nc.sync.dma_start(rowp[t * 128 : (t + 1) * 128, :], scr)

                    # col path per half: starts right after this drain
                    if t == 0:
                        nc.vector.tensor_copy(colacc[:, hs], tsb[:, hs])
                    elif last:
                        q = 1024
                        for jj in range(2):
                            j = half * 2 + jj
                            nc.vector.tensor_tensor(
                                colacc[:, j * q : (j + 1) * q],
                                tsb[:, j * q : (j + 1) * q],
                                colacc[:, j * q : (j + 1) * q],
                                OP.min,
                            )
                            qeng = nc.sync if j % 2 == 0 else nc.gpsimd
                            qeng.dma_start(
                                outc[:, j * q : (j + 1) * q],
                                colacc[:, j * q : (j + 1) * q],
                            )
                    else:
                        nc.vector.tensor_tensor(
                            colacc[:, hs], tsb[:, hs], colacc[:, hs], OP.min
                        )

                # row path for first/last tiles (middle tiles fold above,
                # right after their half-1 drain)
                if t == 0 or last:
                    scr = scr_pool.tile([128, 2048], f16, tag="scr", name="scr")
                    nc.vector.tensor_tensor(
                        scr, tsb[:, 0:2048], tsb[:, 2048:4096], OP.min
                    )
                    eng = nc.gpsimd if last else nc.sync
                    eng.dma_start(rowp[t * 128 : (t + 1) * 128, :], scr)

    nc.compile()
    return nc


def _get(n_rows, m_cols, num_cores):
    key = (n_rows, m_cols, num_cores)
    if key not in _COMPILED:
        _COMPILED[key] = _build(n_rows, m_cols, num_cores)
    return _COMPILED[key]


def _prep(x, y):
    # host-side layout prep: K-major f16 operands with ones/norm rows baked
    n = x.shape[0]
    m = y.shape[0]
    xt = np.empty((KA, n), dtype=np.float16)
    xt[0:K] = (-2.0 * x.T).astype(np.float16)
    xt[K] = 1.0
    xt[K + 1] = (x.astype(np.float64) ** 2).sum(axis=1).astype(np.float16)
    yt = np.empty((KA, m), dtype=np.float16)
    yt[0:K] = y.T.astype(np.float16)
    yt[K] = (y.astype(np.float64) ** 2).sum(axis=1).astype(np.float16)
    yt[K + 1] = 1.0
    return {"xt": np.ascontiguousarray(xt), "yt": np.ascontiguousarray(yt)}


def _run(x, y, n_rows, m_cols, num_cores, trace=False):
    global LAST_RESULTS
    from concourse import bass_utils

    nc = _get(n_rows, m_cols, num_cores)
    in_maps = [_prep(x[b], y[b]) for b in range(num_cores)]
    res = bass_utils.run_bass_kernel_spmd(
        nc, in_maps, core_ids=list(range(num_cores)), trace=trace
    )
    LAST_RESULTS = res
    return [(r["rowp"], r["outc"]) for r in res.results]


def _postprocess(outs):
    total = 0.0
    for rowpart, colacc in outs:
        rmin = rowpart.astype(np.float32).min(axis=1)
        colmin = colacc.astype(np.float32).min(axis=0)
        d1 = np.sqrt(np.maximum(rmin.astype(np.float64), 0.0)).mean()
        d0 = np.sqrt(np.maximum(colmin.astype(np.float64), 0.0)).mean()
        total += d0 + d1
    return np.float32(total / len(outs))


def kernel(input1, input2):
    x = np.asarray(input1, dtype=np.float32)
    y = np.asarray(input2, dtype=np.float32)
    assert x.shape == (B, N, K) and y.shape == (B, M, K), (x.shape, y.shape)
    outs = _run(x, y, N, M, B)
    return _postprocess(outs)


# revision 26
# speedup vs baseline: 1.2226x; 1.0062x over previous
"""Chamfer distance kernel v9 for Trainium2 (Bass/Tile), SPMD over 8 NeuronCores.

~149us HW exec (vs 171us v2 baseline). Structure:
  - Host pre-prep: x/y transposed, scaled (-2x), cast to f16, norm rows
    computed in f64 and baked into the K-major operands (K=66:
    -2x^T|ones|x2 against y^T|y2|ones), so psum holds the full squared
    distance and no phase-0 device work is needed beyond the loads.
  - Inputs loaded in fine-grained column chunks on two DMA queues so the
    first matmuls start ~11us in and never starve.
  - Per tile (32 tiles of 128 x-rows x 4096 y-cols): 8 fp16 matmuls into
    2 psum halves; ACT Copy-drains each half to f16 (the pacing engine,
    ~99.7% busy); DVE does per-half colacc min, plus a fold-min of the two
    halves right after the half-1 drain (row path, shipped 2048-wide to
    the host, which finishes the min/sqrt/mean).
Engine budget per tile: ACT 3.93us, PE ~3.9us (1.2GHz mid p-state,
never ramps), DVE ~3.6us. The f32-psum drain through the 1x ACTIVATE
port is the structural floor on TRN2 (no f16 psum, no min-accumulate,
tensor_tensor_reduce is 1x-only).
"""

import sys

if "/opt/trn_rl_repo" not in sys.path:
    sys.path.insert(0, "/opt/trn_rl_repo")

import numpy as np

B = 8
N = 4096
M = 4096
K = 64
KA = K + 2  # -2x rows | ones | x2   (lhsT);  y rows | y2 | ones  (rhs)
NT = 128    # x rows per tile
MT = 512    # rhs cols per matmul

_COMPILED = {}
LAST_RESULTS = None


def _build(n_rows, m_cols, num_cores):
    import concourse.bacc as bacc
    import concourse.mybir as mybir
    import concourse.tile as tile

    f32 = mybir.dt.float32
    f16 = mybir.dt.float16
    AX = mybir.AxisListType
    OP = mybir.AluOpType

    n_nt = n_rows // NT  # 32 tiles

    nc = bacc.Bacc(
        "TRN2", target_bir_lowering=False, debug=False, num_devices=num_cores
    )
    xtd = nc.dram_tensor("xt", [KA, n_rows], f16, kind="ExternalInput")
    ytd = nc.dram_tensor("yt", [KA, m_cols], f16, kind="ExternalInput")
    rowp = nc.dram_tensor("rowp", [n_rows, 2048], f16, kind="ExternalOutput")
    outc = nc.dram_tensor("outc", [128, m_cols], f16, kind="ExternalOutput")

    with tile.TileContext(nc) as tc:
        with (
            tc.tile_pool(name="const", bufs=1) as cpool,
            tc.tile_pool(name="mpsum", bufs=2, space="PSUM") as ps_pool,
            tc.tile_pool(name="tsbp", bufs=3) as tsb_pool,
            tc.tile_pool(name="scrp", bufs=4) as scr_pool,
        ):
            XT = cpool.tile([KA, n_rows], f16, name="XT")
            YT = cpool.tile([KA, m_cols], f16, name="YT")
            colacc = cpool.tile([128, m_cols], f16, name="colacc")

            # load K-major operands (norm/ones rows prebuilt on host).
            # Fine-grained chunks alternating across two queues so the first
            # matmuls start early and never starve.
            nc.sync.dma_start(YT[:, 0:256], ytd[:, 0:256])
            nc.gpsimd.dma_start(XT[:, 0:1024], xtd[:, 0:1024])
            nc.sync.dma_start(YT[:, 256:1024], ytd[:, 256:1024])
            nc.gpsimd.dma_start(YT[:, 1024:2048], ytd[:, 1024:2048])
            nc.sync.dma_start(YT[:, 2048:3072], ytd[:, 2048:3072])
            nc.gpsimd.dma_start(YT[:, 3072:4096], ytd[:, 3072:4096])
            nc.sync.dma_start(XT[:, 1024:4096], xtd[:, 1024:4096])

            for t in range(n_nt):
                lhsT = XT[:, t * NT : (t + 1) * NT]
                last = t == n_nt - 1
                tsb = tsb_pool.tile([128, m_cols], f16, tag="tsb", name="tsb")
                for half in range(2):
                    ps = ps_pool.tile([128, 2048], f32, tag="ps", name="ps")
                    for h in range(2048 // MT):
                        c0 = half * 2048 + h * MT
                        nc.tensor.matmul(
                            ps[:, h * MT : (h + 1) * MT],
                            lhsT=lhsT,
                            rhs=YT[:, c0 : c0 + MT],
                            start=True,
                            stop=True,
                        )
                    # drain: psum f32 -> sbuf f16 (full squared distance)
                    hs = slice(half * 2048, (half + 1) * 2048)
                    nc.scalar.copy(tsb[:, hs], ps)

                    # after the half-1 drain, fold first so the rowp DMA
                    # starts early; the col h1 update has a full tile of slack
                    if half == 1 and 0 < t < n_nt - 1:
                        scr = scr_pool.tile([128, 2048], f16, tag="scr", name="scr")
                        nc.vector.tensor_tensor(
                            scr, tsb[:, 0:2048], tsb[:, 2048:4096], OP.min
                        )
                        nc.sync.dma_start(rowp[t * 128 : (t + 1) * 128, :], scr)

                    # col path per half: starts right after this drain
                    if t == 0:
                        nc.vector.tensor_copy(colacc[:, hs], tsb[:, hs])
                    elif last:
                        q = 1024
                        for jj in range(2):
                            j = half * 2 + jj
                            nc.vector.tensor_tensor(
                                colacc[:, j * q : (j + 1) * q],
                                tsb[:, j * q : (j + 1) * q],
                                colacc[:, j * q : (j + 1) * q],
                                OP.min,
                            )
                            qeng = nc.sync if j % 2 == 0 else nc.gpsimd
                            qeng.dma_start(
                                outc[:, j * q : (j + 1) * q],
                                colacc[:, j * q : (j + 1) * q],
                            )
                    else:
                        nc.vector.tensor_tensor(
                            colacc[:, hs], tsb[:, hs], colacc[:, hs], OP.min
                        )

                # row path for first/last tiles (middle tiles fold above,
                # right after their half-1 drain)
                if t == 0 or last:
                    scr = scr_pool.tile([128, 2048], f16, tag="scr", name="scr")
                    nc.vector.tensor_tensor(
                        scr, tsb[:, 0:2048], tsb[:, 2048:4096], OP.min
                    )
                    eng = nc.gpsimd if last else nc.sync
                    eng.dma_start(rowp[t * 128 : (t + 1) * 128, :], scr)

    nc.compile()
    return nc


def _get(n_rows, m_cols, num_cores):
    key = (n_rows, m_cols, num_cores)
    if key not in _COMPILED:
        _COMPILED[key] = _build(n_rows, m_cols, num_cores)
    return _COMPILED[key]


def _prep(x, y):
    # host-side layout prep: K-major f16 operands with ones/norm rows baked
    n = x.shape[0]
    m = y.shape[0]
    xt = np.empty((KA, n), dtype=np.float16)
    xt[0:K] = (-2.0 * x.T).astype(np.float16)
    xt[K] = 1.0
    xt[K + 1] = (x.astype(np.float64) ** 2).sum(axis=1).astype(np.float16)
    yt = np.empty((KA, m), dtype=np.float16)
    yt[0:K] = y.T.astype(np.float16)
    yt[K] = (y.astype(np.float64) ** 2).sum(axis=1).astype(np.float16)
    yt[K + 1] = 1.0
    return {"xt": np.ascontiguousarray(xt), "yt": np.ascontiguousarray(yt)}


def _run(x, y, n_rows, m_cols, num_cores, trace=False):
    global LAST_RESULTS
    from concourse import bass_utils

    nc = _get(n_rows, m_cols, num_cores)
    in_maps = [_prep(x[b], y[b]) for b in range(num_cores)]
    res = bass_utils.run_bass_kernel_spmd(
        nc, in_maps, core_ids=list(range(num_cores)), trace=trace
    )
    LAST_RESULTS = res
    return [(r["rowp"], r["outc"]) for r in res.results]


def _postprocess(outs):
    total = 0.0
    for rowpart, colacc in outs:
        rmin = rowpart.astype(np.float32).min(axis=1)
        colmin = colacc.astype(np.float32).min(axis=0)
        d1 = np.sqrt(np.maximum(rmin.astype(np.float64), 0.0)).mean()
        d0 = np.sqrt(np.maximum(colmin.astype(np.float64), 0.0)).mean()
        total += d0 + d1
    return np.float32(total / len(outs))


def kernel(input1, input2):
    x = np.asarray(input1, dtype=np.float32)
    y = np.asarray(input2, dtype=np.float32)
    assert x.shape == (B, N, K) and y.shape == (B, M, K), (x.shape, y.shape)
    outs = _run(x, y, N, M, B)
    return _postprocess(outs)
